# revision 28
# baseline (speedup 1.0000x reference)
"""Trainium2 Bass kernel for nn_LinearCondensed.

Computes out[b, o] = sum_k weight[o, k] * x[b, indx_seqs[o, k]] + bias[o]
with B=2048, IN_F=OUT_F=4096, FAN_IN=32.

Strategy: the gather has no fast on-chip primitive (GPSIMD ap_gather measured
~20x below its modeled rate; DMA descriptor gather materializes 32x the data
of x), so we densify the sparse weight matrix on the host --
W'[o, i] = sum_{k: indx_seqs[o,k]==i} weight[o, k] -- and run a dense bf16
matmul out = x @ W'^T + bias on the PE array (1 cycle/row, same as fp32r,
but half the DMA traffic; measured rel_err 3.0e-3 vs the 2e-2 gate; fp8
DoubleRow would be 2x PE but fails the gate at 3-5e-2). OUT_F is sharded
8 ways across cores (512 columns each), x replicated. The kernel is
PE-bound (~110us of streaming at 512 rows/matmul); the single sync HWDGE
queue sustains ~390 GB/s, which keeps every dependency ahead of the PE:
x0, x1, then W in 8 groups (first split 1+3) pace the k-outer phase over
b-tiles 0-1, and x2+ stream during the k-inner phase. Dummy matmuls fill
the ~7us engine-boot head so the PE p-state is fully ramped when real work
arrives; the last b-tile accumulates in two half-width PSUM groups so its
drain overlaps its final matmuls. Bias is folded into the PSUM drain
(pre-replicated across partitions on host). Host pre-tiles both operands
into the exact SBUF layouts so every DMA is a large contiguous copy.
"""

import os
import sys
import types

import ml_dtypes
import numpy as np

import concourse.bacc as bacc
import concourse.mybir as mybir
import concourse.tile as tile
from concourse.bass_utils import run_bass_kernel_spmd

B, IN_F, OUT_F, FAN_IN = 2048, 4096, 4096, 32
NCORES = 8
OSH = OUT_F // NCORES          # 512 output features per core
P = 128                        # partitions
BT = B // P                    # 16 batch tiles
KT = IN_F // P                 # 32 contraction tiles
N = OSH                        # 512 moving columns (max for fp32)

f32 = mybir.dt.float32
f32r = mybir.dt.float32r
bf16 = mybir.dt.bfloat16

_cache = {}


def _enable_ntff_hook():
    """Register the ctypes NTFF profile hook (the image's antenv lacks
    axon_hooks); lets trace=True produce a neuron-profile under axon."""
    try:
        from antenv.axon_hooks import get_axon_ntff_profile_hook  # noqa: F401
        return
    except ImportError:
        pass
    try:
        import antenv
        from trn_agent_boot.trn_boot import _ntff_profile_via_ctypes

        mod = types.ModuleType("antenv.axon_hooks")
        holder = [None]
        mod.set_axon_ntff_profile_hook = lambda h: holder.__setitem__(0, h)
        mod.get_axon_ntff_profile_hook = lambda: holder[0]
        antenv.axon_hooks = mod
        sys.modules["antenv.axon_hooks"] = mod
        mod.set_axon_ntff_profile_hook(
            _ntff_profile_via_ctypes("/opt/axon/libaxon_pjrt.so"))
        import concourse.bass_utils as bu
        bu.upload_artifacts = lambda tmpdir: str(tmpdir)
    except Exception:
        pass


def _build():
    nc = bacc.Bacc()
    # xt[t] is the (128p=i-within-ktile, KT*128=b columns... see layout below)
    # Layouts (host-pretiled, all contiguous):
    #   XT[t, p, a, c] = x[t*128 + c, a*128 + p]   -> per b-tile t: [128, KT*128]
    #   WT[p, a, n]    = W'[o0 + n, a*128 + p]     -> [128, KT*512]
    XT = nc.declare_dram_parameter("XT", [BT, P, KT * P], bf16, isOutput=False)
    WT = nc.declare_dram_parameter("WT", [KT, P, N], bf16, isOutput=False)
    BIAS = nc.declare_dram_parameter("BIAS", [P, N], f32, isOutput=False)
    OUT = nc.declare_dram_parameter("OUT", [B, N], f32, isOutput=True)

    XTv = XT.ap().rearrange("t p (a c) -> t p a c", a=KT)

    with tile.TileContext(nc) as tc:
        with (
            tc.tile_pool(name="wpool", bufs=1) as wpool,
            tc.tile_pool(name="xpool", bufs=3) as xpool,
            tc.tile_pool(name="cpool", bufs=1) as cpool,
            tc.tile_pool(name="opool", bufs=2) as opool,
            tc.tile_pool(name="psum", bufs=4, space="PSUM") as psum,
        ):
            # All input loads ride the single sync HWDGE FIFO in a deliberate
            # order: x0, x1 at full bandwidth (PE can start at ~6us), then
            # the 32 weight k-tiles (which pace b-tile 0), then x2+ arrive
            # just in time. Output stores use the scalar HWDGE queue so they
            # never block input loads.
            xtiles = {}

            # The PE p-state ramps to 2.4GHz only after ~3us of continuous
            # work; real matmuls can't start until x0+x1+wg0 land (~15us).
            # Fill the idle head with dummy matmuls on zeroed SBUF so the
            # array is warm (and the pipeline primed) when real work arrives.
            dl = cpool.tile([P, P], bf16)
            dr = cpool.tile([P, N], bf16)
            nc.vector.memset(dl[:], 0)
            nc.vector.memset(dr[:], 0)
            dacc = psum.tile([P, N], f32, name="dacc", tag="dacc", bufs=1)
            for _ in range(20):
                nc.tensor.matmul(dacc[:], dl[:], dr[:], start=True, stop=True)

            def load_x(t):
                xs = xpool.tile([P, KT, P], bf16, tag="xs")
                nc.sync.dma_start(xs[:], XTv[t])
                xtiles[t] = xs

            load_x(0)
            load_x(1)
            # All 32 W k-tiles live in ONE tile (fewer tile tags -> fewer
            # semaphores -> shorter end-of-kernel sem-clear sweep, which is
            # most of the drain tail). DMA chunk sizes ramp 1,1,2,2,2,4...
            # so each chunk's semaphore fires just ahead of the PE's
            # 0.43us/k-tile consumption (a single 4-k-tile DMA would sem
            # 0.7us after k-tile 1 is needed); subtile dependency tracking
            # maps each matmul to its own chunk's semaphore.
            WTall = WT.ap().rearrange("k p n -> p k n")
            wall = wpool.tile([P, KT, N], bf16)
            brow = None
            for (a0, a1) in ((0, 1), (1, 2), (2, 4), (4, 6), (6, 8), (8, 12),
                             (12, 16), (16, 20), (20, 24), (24, 28), (28, 32)):
                nc.sync.dma_start(wall[:, a0:a1, :], WTall[:, a0:a1, :])
                if a0 == 4:
                    brow = cpool.tile([P, N], f32)
                    nc.sync.dma_start(brow[:], BIAS[:])
            wtiles = [wall[:, a, :] for a in range(KT)]

            # bias folded into the PSUM drain: osb = acc + bias (bias row
            # pre-replicated across partitions on host), saving 16 K=1 bias
            # matmuls on the PE.
            def finish_tile(t, acc):
                osb = opool.tile([P, N], f32, tag="osb")
                nc.vector.tensor_tensor(osb[:], acc[:], brow[:], mybir.AluOpType.add)
                nc.scalar.dma_start(OUT.ap()[t * P:(t + 1) * P, :], osb[:])

            # Phase 1: b-tiles 0-1 in k-outer order so the PE consumes each
            # weight group as it lands instead of idling through the 8MB
            # weight stream.
            G = 2
            accs = [psum.tile([P, N], f32, name=f"acc{t}", tag="acc")
                    for t in range(G)]
            for a in range(KT):
                for t in range(G):
                    nc.tensor.matmul(
                        accs[t][:], xtiles[t][:, a, :], wtiles[a][:],
                        start=(a == 0), stop=(a == KT - 1),
                    )
            for t in range(G):
                finish_tile(t, accs[t])

            # Phase 2: remaining b-tiles, k-inner, x streamed just in time.
            for t in range(G, BT - 1):
                load_x(t)
                xsb = xtiles[t]
                acc = psum.tile([P, N], f32, tag="acc")
                for a in range(KT):
                    nc.tensor.matmul(
                        acc[:],
                        xsb[:, a, :],      # lhsT: [K=128 (i), M=128 (b)]
                        wtiles[a][:],      # rhs:  [K=128 (i), N=512 (o)]
                        start=(a == 0),
                        stop=(a == KT - 1),
                    )
                finish_tile(t, acc)

            # Last b-tile: split into two 256-column accumulation groups so
            # the first half's bias-add + store overlap the second half's
            # final matmuls, shortening the drain tail after the last matmul.
            t = BT - 1
            load_x(t)
            xsb = xtiles[t]
            H = N // 2
            acc_h = [psum.tile([P, H], f32, name=f"acch{h}", tag="acch", bufs=2)
                     for h in range(2)]
            for a in range(KT):
                for h in range(2):
                    nc.tensor.matmul(
                        acc_h[h][:], xsb[:, a, :],
                        wtiles[a][:, h * H:(h + 1) * H],
                        start=(a == 0), stop=(a == KT - 1),
                    )
            for h in range(2):
                osb = opool.tile([P, H], f32, tag=f"osbh{h}")
                nc.vector.tensor_tensor(
                    osb[:], acc_h[h][:], brow[:, h * H:(h + 1) * H],
                    mybir.AluOpType.add)
                nc.scalar.dma_start(
                    OUT.ap()[t * P:(t + 1) * P, h * H:(h + 1) * H], osb[:])

    nc.compile()
    return nc


def kernel(x, weight, bias, indx_seqs):
    x = np.asarray(x, dtype=np.float32)
    weight = np.asarray(weight, dtype=np.float32)
    bias = np.asarray(bias, dtype=np.float32)
    indx_seqs = np.asarray(indx_seqs)

    if "nc" not in _cache:
        _cache["nc"] = _build()
    nc = _cache["nc"]

    # Densify sparse weights: W'[o, i] += weight[o, k] at i = indx_seqs[o, k]
    wd = np.zeros((OUT_F, IN_F), dtype=np.float32)
    np.add.at(wd, (np.arange(OUT_F)[:, None], indx_seqs), weight)

    # Host pre-tiling into SBUF-friendly layouts, cast to bf16 (the PE runs
    # bf16 at the same 1 cycle/row as fp32r, so this halves DMA traffic at a
    # measured cost of rel_err 3.0e-3 vs the 2e-2 gate).
    # XT[t, p, a, c] = x[t*128+c, a*128+p]
    xt = np.ascontiguousarray(
        x.reshape(BT, P, KT, P).transpose(0, 3, 2, 1)
    ).reshape(BT, P, KT * P).astype(ml_dtypes.bfloat16)
    in_maps = []
    for c in range(NCORES):
        wshard = wd[c * OSH:(c + 1) * OSH]            # (512, 4096)
        # WT[a, p, n] = W'[o0+n, a*128+p]
        wt = np.ascontiguousarray(
            wshard.reshape(OSH, KT, P).transpose(1, 2, 0)).astype(ml_dtypes.bfloat16)
        in_maps.append({
            "XT": xt,
            "WT": wt,
            "BIAS": np.ascontiguousarray(np.broadcast_to(bias[c * OSH:(c + 1) * OSH], (P, N))),
        })

    trace = bool(int(os.environ.get("BASSK_TRACE", "0"))) or bool(
        os.environ.get("BASS_TRACE"))
    if trace:
        _enable_ntff_hook()
    res = run_bass_kernel_spmd(
        nc, in_maps, list(range(NCORES)), trace=trace,
        trace_cores=list(range(NCORES)) if trace else None,
    )
    _cache["last_results"] = res

    out = np.concatenate([res.results[c]["OUT"] for c in range(NCORES)], axis=1)
    return out



# revision 31
# speedup vs baseline: 1.0275x; 1.0275x over previous
"""Trainium2 Bass kernel for nn_LinearCondensed.

Computes out[b, o] = sum_k weight[o, k] * x[b, indx_seqs[o, k]] + bias[o]
with B=2048, IN_F=OUT_F=4096, FAN_IN=32.

Strategy: the gather has no fast on-chip primitive (GPSIMD ap_gather measured
~20x below its modeled rate; DMA descriptor gather materializes 32x the data
of x), so we densify the sparse weight matrix on the host --
W'[o, i] = sum_{k: indx_seqs[o,k]==i} weight[o, k] -- and run a dense bf16
matmul out = x @ W'^T + bias on the PE array (1 cycle/row, same as fp32r,
but half the DMA traffic; measured rel_err 3.0e-3 vs the 2e-2 gate; fp8
DoubleRow would be 2x PE but fails the gate at 3-5e-2). OUT_F is sharded
8 ways across cores (512 columns each), x replicated. The kernel is
PE-bound (~110us of streaming at 512 rows/matmul); the single sync HWDGE
queue sustains ~390 GB/s, which keeps every dependency ahead of the PE:
x0, x1, then W in 8 groups (first split 1+3) pace the k-outer phase over
b-tiles 0-1, and x2+ stream during the k-inner phase. Dummy matmuls fill
the ~7us engine-boot head so the PE p-state is fully ramped when real work
arrives; the last b-tile accumulates in two half-width PSUM groups so its
drain overlaps its final matmuls. Bias is folded into the PSUM drain
(pre-replicated across partitions on host). Host pre-tiles both operands
into the exact SBUF layouts so every DMA is a large contiguous copy.
"""

import os
import sys
import types

import ml_dtypes
import numpy as np

import concourse.bacc as bacc
import concourse.mybir as mybir
import concourse.tile as tile
from concourse.bass_utils import run_bass_kernel_spmd

B, IN_F, OUT_F, FAN_IN = 2048, 4096, 4096, 32
NCORES = 8
OSH = OUT_F // NCORES          # 512 output features per core
P = 128                        # partitions
BT = B // P                    # 16 batch tiles
KT = IN_F // P                 # 32 contraction tiles
N = OSH                        # 512 moving columns (max for fp32)

f32 = mybir.dt.float32
f32r = mybir.dt.float32r
bf16 = mybir.dt.bfloat16

_cache = {}


def _enable_ntff_hook():
    """Register the ctypes NTFF profile hook (the image's antenv lacks
    axon_hooks); lets trace=True produce a neuron-profile under axon."""
    try:
        from antenv.axon_hooks import get_axon_ntff_profile_hook  # noqa: F401
        return
    except ImportError:
        pass
    try:
        import antenv
        from trn_agent_boot.trn_boot import _ntff_profile_via_ctypes

        mod = types.ModuleType("antenv.axon_hooks")
        holder = [None]
        mod.set_axon_ntff_profile_hook = lambda h: holder.__setitem__(0, h)
        mod.get_axon_ntff_profile_hook = lambda: holder[0]
        antenv.axon_hooks = mod
        sys.modules["antenv.axon_hooks"] = mod
        mod.set_axon_ntff_profile_hook(
            _ntff_profile_via_ctypes("/opt/axon/libaxon_pjrt.so"))
        import concourse.bass_utils as bu
        bu.upload_artifacts = lambda tmpdir: str(tmpdir)
    except Exception:
        pass


def _build():
    nc = bacc.Bacc()
    # Layouts (host-pretiled, all contiguous):
    #   XT[t, p, a, c]  = x[t*128 + c, a*128 + p]  -> per b-tile t: [128, KT*128]
    #   PH1[p, a, :]    = [x0 | x1 | x2 | w] per k-tile: the phase-1 stream
    #                     pre-interleaved in PE consumption order, so one
    #                     ramped chunk sequence of large DMAs (trigger cost
    #                     ~0.95us each caps us at ~11 loads) lets the PE
    #                     start at ~9.5us instead of idling through a serial
    #                     x0+x1 prefix until 14.5us.
    GP1 = 3                     # b-tiles covered by phase 1
    XW = GP1 * P + N            # 896 elements per (partition, k-tile)
    PH1 = nc.declare_dram_parameter("PH1", [P, KT, XW], bf16, isOutput=False)
    XT = nc.declare_dram_parameter("XT", [BT, P, KT * P], bf16, isOutput=False)
    BIAS = nc.declare_dram_parameter("BIAS", [P, N], f32, isOutput=False)
    OUT = nc.declare_dram_parameter("OUT", [B, N], f32, isOutput=True)

    XTv = XT.ap().rearrange("t p (a c) -> t p a c", a=KT)

    with tile.TileContext(nc) as tc:
        with (
            tc.tile_pool(name="wpool", bufs=1) as wpool,
            tc.tile_pool(name="xpool", bufs=4) as xpool,
            tc.tile_pool(name="cpool", bufs=1) as cpool,
            tc.tile_pool(name="opool", bufs=3) as opool,
            tc.tile_pool(name="psum", bufs=4, space="PSUM") as psum,
        ):
            xtiles = {}

            # Short PE warmup: phase 1 now starts at ~9.5us, so only a few
            # dummies fit before real work (p-state finishes ramping during
            # the stream-paced early k-tiles).
            dl = cpool.tile([P, P], bf16)
            dr = cpool.tile([P, N], bf16)
            nc.vector.memset(dl[:], 0)
            nc.vector.memset(dr[:], 0)
            dacc = psum.tile([P, N], f32, name="dacc", tag="dacc", bufs=1)
            for _ in range(5):
                nc.tensor.matmul(dacc[:], dl[:], dr[:], start=True, stop=True)

            # Phase-1 stream: ramped k-tile chunks; each chunk's semaphore
            # fires just ahead of the PE's consumption, and subtile
            # dependency tracking maps each matmul to its own chunk.
            ph1 = wpool.tile([P, KT, XW], bf16)
            brow = None
            for (a0, a1) in ((0, 1), (1, 2), (2, 4), (4, 8), (8, 12),
                             (12, 16), (16, 20), (20, 24), (24, 28), (28, 32)):
                nc.sync.dma_start(ph1[:, a0:a1, :], PH1.ap()[:, a0:a1, :])
                if a0 == 8:
                    brow = cpool.tile([P, N], f32)
                    nc.sync.dma_start(brow[:], BIAS[:])
            wtiles = [ph1[:, a, GP1 * P:] for a in range(KT)]

            def load_x(t):
                xs = xpool.tile([P, KT, P], bf16, tag="xs")
                nc.sync.dma_start(xs[:], XTv[t])
                xtiles[t] = xs

            # bias folded into the PSUM drain (bias row pre-replicated
            # across partitions on host)
            def finish_tile(t, acc):
                osb = opool.tile([P, N], f32, tag="osb")
                nc.vector.tensor_tensor(osb[:], acc[:], brow[:], mybir.AluOpType.add)
                nc.scalar.dma_start(OUT.ap()[t * P:(t + 1) * P, :], osb[:])

            # Phase 1: b-tiles 0-2 k-outer, fed directly from the
            # interleaved stream.
            accs = [psum.tile([P, N], f32, name=f"acc{t}", tag="acc")
                    for t in range(GP1)]
            for a in range(KT):
                for t in range(GP1):
                    nc.tensor.matmul(
                        accs[t][:], ph1[:, a, t * P:(t + 1) * P], wtiles[a][:],
                        start=(a == 0), stop=(a == KT - 1),
                    )
            for t in range(GP1):
                finish_tile(t, accs[t])

            # Phase 2: remaining b-tiles, k-inner, x streamed just in time.
            for t in range(GP1, BT - 1):
                load_x(t)
                xsb = xtiles[t]
                acc = psum.tile([P, N], f32, tag="acc")
                for a in range(KT):
                    nc.tensor.matmul(
                        acc[:],
                        xsb[:, a, :],      # lhsT: [K=128 (i), M=128 (b)]
                        wtiles[a][:],      # rhs:  [K=128 (i), N=512 (o)]
                        start=(a == 0),
                        stop=(a == KT - 1),
                    )
                finish_tile(t, acc)

            # Last b-tile: two half-width accumulation groups so the first
            # half's bias-add + store overlap the second half's final
            # matmuls, shortening the drain tail after the last matmul.
            t = BT - 1
            load_x(t)
            xsb = xtiles[t]
            H = N // 2
            acc_h = [psum.tile([P, H], f32, name=f"acch{h}", tag="acch", bufs=2)
                     for h in range(2)]
            for a in range(KT):
                for h in range(2):
                    nc.tensor.matmul(
                        acc_h[h][:], xsb[:, a, :],
                        wtiles[a][:, h * H:(h + 1) * H],
                        start=(a == 0), stop=(a == KT - 1),
                    )
            for h in range(2):
                osb = opool.tile([P, H], f32, tag=f"osbh{h}")
                nc.vector.tensor_tensor(
                    osb[:], acc_h[h][:], brow[:, h * H:(h + 1) * H],
                    mybir.AluOpType.add)
                nc.scalar.dma_start(
                    OUT.ap()[t * P:(t + 1) * P, h * H:(h + 1) * H], osb[:])

    nc.compile()
    return nc


def kernel(x, weight, bias, indx_seqs):
    x = np.asarray(x, dtype=np.float32)
    weight = np.asarray(weight, dtype=np.float32)
    bias = np.asarray(bias, dtype=np.float32)
    indx_seqs = np.asarray(indx_seqs)

    if "nc" not in _cache:
        _cache["nc"] = _build()
    nc = _cache["nc"]

    # Densify sparse weights: W'[o, i] += weight[o, k] at i = indx_seqs[o, k]
    wd = np.zeros((OUT_F, IN_F), dtype=np.float32)
    np.add.at(wd, (np.arange(OUT_F)[:, None], indx_seqs), weight)

    # Host pre-tiling into SBUF-friendly layouts, cast to bf16 (the PE runs
    # bf16 at the same 1 cycle/row as fp32r, so this halves DMA traffic at a
    # measured cost of rel_err 3.0e-3 vs the 2e-2 gate).
    # XT[t, p, a, c] = x[t*128+c, a*128+p]
    xt = np.ascontiguousarray(
        x.reshape(BT, P, KT, P).transpose(0, 3, 2, 1)
    ).reshape(BT, P, KT * P).astype(ml_dtypes.bfloat16)
    in_maps = []
    for c in range(NCORES):
        wshard = wd[c * OSH:(c + 1) * OSH]            # (512, 4096)
        # WT[a, p, n] = W'[o0+n, a*128+p]
        wt = np.ascontiguousarray(
            wshard.reshape(OSH, KT, P).transpose(1, 2, 0)).astype(ml_dtypes.bfloat16)
        # PH1[p, a, :] = [x0 | x1 | x2 | w] per k-tile (phase-1 stream in
        # consumption order; b-tiles 0-2 + the full W shard)
        ph1 = np.concatenate(
            [xt[t].reshape(P, KT, P) for t in range(3)]
            + [wt.transpose(1, 0, 2)], axis=2)
        in_maps.append({
            "PH1": np.ascontiguousarray(ph1),
            "XT": xt,
            "BIAS": np.ascontiguousarray(np.broadcast_to(bias[c * OSH:(c + 1) * OSH], (P, N))),
        })
    trace = bool(int(os.environ.get("BASSK_TRACE", "0"))) or bool(
        os.environ.get("BASS_TRACE"))
    if trace:
        _enable_ntff_hook()
    res = run_bass_kernel_spmd(
        nc, in_maps, list(range(NCORES)), trace=trace,
        trace_cores=list(range(NCORES)) if trace else None,
    )
    _cache["last_results"] = res

    out = np.concatenate([res.results[c]["OUT"] for c in range(NCORES)], axis=1)
    return out



# revision 32
# speedup vs baseline: 1.0458x; 1.0179x over previous
"""Trainium2 Bass kernel for nn_LinearCondensed.

Computes out[b, o] = sum_k weight[o, k] * x[b, indx_seqs[o, k]] + bias[o]
with B=2048, IN_F=OUT_F=4096, FAN_IN=32.

Strategy: the gather has no fast on-chip primitive (GPSIMD ap_gather measured
~20x below its modeled rate; DMA descriptor gather materializes 32x the data
of x), so we densify the sparse weight matrix on the host --
W'[o, i] = sum_{k: indx_seqs[o,k]==i} weight[o, k] -- and run a dense bf16
matmul out = x @ W'^T + bias on the PE array (1 cycle/row, same as fp32r,
but half the DMA traffic; measured rel_err 3.0e-3 vs the 2e-2 gate; fp8
DoubleRow would be 2x PE but fails the gate at 3-5e-2). OUT_F is sharded
8 ways across cores (512 columns each), x replicated. The kernel is
PE-bound (~110us of streaming at 512 rows/matmul); the single sync HWDGE
queue sustains ~390 GB/s, which keeps every dependency ahead of the PE:
x0, x1, then W in 8 groups (first split 1+3) pace the k-outer phase over
b-tiles 0-1, and x2+ stream during the k-inner phase. Dummy matmuls fill
the ~7us engine-boot head so the PE p-state is fully ramped when real work
arrives; the last b-tile accumulates in two half-width PSUM groups so its
drain overlaps its final matmuls. Bias is folded into the PSUM drain
(pre-replicated across partitions on host). Host pre-tiles both operands
into the exact SBUF layouts so every DMA is a large contiguous copy.
"""

import os
import sys
import types

import ml_dtypes
import numpy as np

import concourse.bacc as bacc
import concourse.mybir as mybir
import concourse.tile as tile
from concourse.bass_utils import run_bass_kernel_spmd

B, IN_F, OUT_F, FAN_IN = 2048, 4096, 4096, 32
NCORES = 8
OSH = OUT_F // NCORES          # 512 output features per core
P = 128                        # partitions
BT = B // P                    # 16 batch tiles
KT = IN_F // P                 # 32 contraction tiles
N = OSH                        # 512 moving columns (max for fp32)

f32 = mybir.dt.float32
f32r = mybir.dt.float32r
bf16 = mybir.dt.bfloat16

_cache = {}


def _enable_ntff_hook():
    """Register the ctypes NTFF profile hook (the image's antenv lacks
    axon_hooks); lets trace=True produce a neuron-profile under axon."""
    try:
        from antenv.axon_hooks import get_axon_ntff_profile_hook  # noqa: F401
        return
    except ImportError:
        pass
    try:
        import antenv
        from trn_agent_boot.trn_boot import _ntff_profile_via_ctypes

        mod = types.ModuleType("antenv.axon_hooks")
        holder = [None]
        mod.set_axon_ntff_profile_hook = lambda h: holder.__setitem__(0, h)
        mod.get_axon_ntff_profile_hook = lambda: holder[0]
        antenv.axon_hooks = mod
        sys.modules["antenv.axon_hooks"] = mod
        mod.set_axon_ntff_profile_hook(
            _ntff_profile_via_ctypes("/opt/axon/libaxon_pjrt.so"))
        import concourse.bass_utils as bu
        bu.upload_artifacts = lambda tmpdir: str(tmpdir)
    except Exception:
        pass


def _build():
    nc = bacc.Bacc()
    # Layouts (host-pretiled, all contiguous):
    #   XT[t, p, a, c]  = x[t*128 + c, a*128 + p]  -> per b-tile t: [128, KT*128]
    #   PH1[p, a, :]    = [x0 | x1 | x2 | w] per k-tile: the phase-1 stream
    #                     pre-interleaved in PE consumption order, so one
    #                     ramped chunk sequence of large DMAs (trigger cost
    #                     ~0.95us each caps us at ~11 loads) lets the PE
    #                     start at ~9.5us instead of idling through a serial
    #                     x0+x1 prefix until 14.5us.
    GP1 = 3                     # b-tiles covered by phase 1
    XW = GP1 * P + N            # 896 elements per (partition, k-tile)
    PH1 = nc.declare_dram_parameter("PH1", [P, KT, XW], bf16, isOutput=False)
    XT = nc.declare_dram_parameter("XT", [BT, P, KT * P], bf16, isOutput=False)
    BIAS = nc.declare_dram_parameter("BIAS", [P, N], f32, isOutput=False)
    OUT = nc.declare_dram_parameter("OUT", [B, N], f32, isOutput=True)

    XTv = XT.ap().rearrange("t p (a c) -> t p a c", a=KT)

    with tile.TileContext(nc) as tc:
        with (
            tc.tile_pool(name="wpool", bufs=1) as wpool,
            tc.tile_pool(name="xpool", bufs=4) as xpool,
            tc.tile_pool(name="cpool", bufs=1) as cpool,
            tc.tile_pool(name="opool", bufs=3) as opool,
            tc.tile_pool(name="psum", bufs=4, space="PSUM") as psum,
        ):
            xtiles = {}

            # Short PE warmup: phase 1 now starts at ~9.5us, so only a few
            # dummies fit before real work (p-state finishes ramping during
            # the stream-paced early k-tiles).
            dl = cpool.tile([P, P], bf16)
            dr = cpool.tile([P, N], bf16)
            nc.vector.memset(dl[:], 0)
            nc.vector.memset(dr[:], 0)
            dacc = psum.tile([P, N], f32, name="dacc", tag="dacc", bufs=1)
            for _ in range(5):
                nc.tensor.matmul(dacc[:], dl[:], dr[:], start=True, stop=True)

            # Phase-1 stream: ramped k-tile chunks; each chunk's semaphore
            # fires just ahead of the PE's consumption, and subtile
            # dependency tracking maps each matmul to its own chunk.
            ph1 = wpool.tile([P, KT, XW], bf16)
            brow = None
            # chunk ramp 2,2,2,2,4...: each ~0.955us DMA trigger must buy
            # >=1.5 k-tiles of PE work (0.648us/k-tile) to outrun the
            # trigger cadence; cumulative k-tiles stay >= 1.5*chunk_index+1.5
            for (a0, a1) in ((0, 2), (2, 4), (4, 6), (6, 8), (8, 12),
                             (12, 16), (16, 20), (20, 24), (24, 28), (28, 32)):
                nc.sync.dma_start(ph1[:, a0:a1, :], PH1.ap()[:, a0:a1, :])
                if a0 == 8:
                    brow = cpool.tile([P, N], f32)
                    nc.sync.dma_start(brow[:], BIAS[:])
            wtiles = [ph1[:, a, GP1 * P:] for a in range(KT)]

            def load_x(t):
                xs = xpool.tile([P, KT, P], bf16, tag="xs")
                nc.sync.dma_start(xs[:], XTv[t])
                xtiles[t] = xs

            # bias folded into the PSUM drain (bias row pre-replicated
            # across partitions on host)
            def finish_tile(t, acc):
                osb = opool.tile([P, N], f32, tag="osb")
                nc.vector.tensor_tensor(osb[:], acc[:], brow[:], mybir.AluOpType.add)
                nc.scalar.dma_start(OUT.ap()[t * P:(t + 1) * P, :], osb[:])

            # Phase 1: b-tiles 0-2 k-outer, fed directly from the
            # interleaved stream.
            accs = [psum.tile([P, N], f32, name=f"acc{t}", tag="acc")
                    for t in range(GP1)]
            for a in range(KT):
                for t in range(GP1):
                    nc.tensor.matmul(
                        accs[t][:], ph1[:, a, t * P:(t + 1) * P], wtiles[a][:],
                        start=(a == 0), stop=(a == KT - 1),
                    )
            for t in range(GP1):
                finish_tile(t, accs[t])

            # Phase 2: remaining b-tiles, k-inner, x streamed just in time.
            for t in range(GP1, BT - 1):
                load_x(t)
                xsb = xtiles[t]
                acc = psum.tile([P, N], f32, tag="acc")
                for a in range(KT):
                    nc.tensor.matmul(
                        acc[:],
                        xsb[:, a, :],      # lhsT: [K=128 (i), M=128 (b)]
                        wtiles[a][:],      # rhs:  [K=128 (i), N=512 (o)]
                        start=(a == 0),
                        stop=(a == KT - 1),
                    )
                finish_tile(t, acc)

            # Last b-tile: two half-width accumulation groups so the first
            # half's bias-add + store overlap the second half's final
            # matmuls, shortening the drain tail after the last matmul.
            t = BT - 1
            load_x(t)
            xsb = xtiles[t]
            H = N // 2
            acc_h = [psum.tile([P, H], f32, name=f"acch{h}", tag="acch", bufs=2)
                     for h in range(2)]
            for a in range(KT):
                for h in range(2):
                    nc.tensor.matmul(
                        acc_h[h][:], xsb[:, a, :],
                        wtiles[a][:, h * H:(h + 1) * H],
                        start=(a == 0), stop=(a == KT - 1),
                    )
            for h in range(2):
                osb = opool.tile([P, H], f32, tag=f"osbh{h}")
                nc.vector.tensor_tensor(
                    osb[:], acc_h[h][:], brow[:, h * H:(h + 1) * H],
                    mybir.AluOpType.add)
                nc.scalar.dma_start(
                    OUT.ap()[t * P:(t + 1) * P, h * H:(h + 1) * H], osb[:])

    nc.compile()
    return nc


def kernel(x, weight, bias, indx_seqs):
    x = np.asarray(x, dtype=np.float32)
    weight = np.asarray(weight, dtype=np.float32)
    bias = np.asarray(bias, dtype=np.float32)
    indx_seqs = np.asarray(indx_seqs)

    if "nc" not in _cache:
        _cache["nc"] = _build()
    nc = _cache["nc"]

    # Densify sparse weights: W'[o, i] += weight[o, k] at i = indx_seqs[o, k]
    wd = np.zeros((OUT_F, IN_F), dtype=np.float32)
    np.add.at(wd, (np.arange(OUT_F)[:, None], indx_seqs), weight)

    # Host pre-tiling into SBUF-friendly layouts, cast to bf16 (the PE runs
    # bf16 at the same 1 cycle/row as fp32r, so this halves DMA traffic at a
    # measured cost of rel_err 3.0e-3 vs the 2e-2 gate).
    # XT[t, p, a, c] = x[t*128+c, a*128+p]
    xt = np.ascontiguousarray(
        x.reshape(BT, P, KT, P).transpose(0, 3, 2, 1)
    ).reshape(BT, P, KT * P).astype(ml_dtypes.bfloat16)
    in_maps = []
    for c in range(NCORES):
        wshard = wd[c * OSH:(c + 1) * OSH]            # (512, 4096)
        # WT[a, p, n] = W'[o0+n, a*128+p]
        wt = np.ascontiguousarray(
            wshard.reshape(OSH, KT, P).transpose(1, 2, 0)).astype(ml_dtypes.bfloat16)
        # PH1[p, a, :] = [x0 | x1 | x2 | w] per k-tile (phase-1 stream in
        # consumption order; b-tiles 0-2 + the full W shard)
        ph1 = np.concatenate(
            [xt[t].reshape(P, KT, P) for t in range(3)]
            + [wt.transpose(1, 0, 2)], axis=2)
        in_maps.append({
            "PH1": np.ascontiguousarray(ph1),
            "XT": xt,
            "BIAS": np.ascontiguousarray(np.broadcast_to(bias[c * OSH:(c + 1) * OSH], (P, N))),
        })
    trace = bool(int(os.environ.get("BASSK_TRACE", "0"))) or bool(
        os.environ.get("BASS_TRACE"))
    if trace:
        _enable_ntff_hook()
    res = run_bass_kernel_spmd(
        nc, in_maps, list(range(NCORES)), trace=trace,
        trace_cores=list(range(NCORES)) if trace else None,
    )
    _cache["last_results"] = res

    out = np.concatenate([res.results[c]["OUT"] for c in range(NCORES)], axis=1)
    return out

